# revision 42
# baseline (speedup 1.0000x reference)
"""GNN ensemble MoE-routing kernel for Trainium2 (8 NeuronCores).

Reference computes all 8 expert MLPs for every sample then selects one
(8x wasted FLOPs). This kernel routes on the host instead: samples are
gathered per expert, and core c runs ONLY expert c's MLP over the
samples routed to it (expert-parallel sharding).

Math folding (exact):
  lat = eps*sigma_c + mu_c  =>  lat @ W1_c = eps @ (sigma_c*W1_c) + mu_c@W1_c
so the device computes  sigmoid(eps @ W1p + b1p) @ W2 + b2  with
  W1p = sigma_c * W1_c,  b1p = b1_c + mu_c @ W1_c  (folded on host).

Precision/perf layout (rel-err budget 2e-2):
  Both matmuls run in fp8 e4m3 with perf_mode=DoubleRow (2 contraction
  rows packed per PE cell -> half the matmul count of bf16). eps
  quantizes to e4m3 raw (std 1.0); W1p is scaled x64 before
  quantization so its ~0.03-std values stay in e4m3's normal range.
  For mm2, sigmoid(x) = 0.5 + 0.5*tanh(x/2): the scalar engine emits
  t = tanh(x/2) in [-1,1] straight to fp8 (the 1/(2*64) rescale folds
  into the activation's scale operand, b1p/2 into its bias), the exact
  0.5*colsum(W2) mean term folds into the output bias on the host, and
  the device contracts t against W2/2 (scaled x64 into e4m3 range).
  Centering means fp8 noise hits only the sigmoid's deviation from
  0.5, not its mean. y = ps2/64 + bias2 via a fused DVE mult+add,
  written back as bf16.

Device layout: features on SBUF partitions, samples on the free axis.
DoubleRow operands are [128, K_blocks, free] with contraction index
k = block*128 + partition; host tensors pack as
reshape(blocks,128,cols).transpose(1,0,2) -> contiguous multi-KB DMA
descriptors. Chunk 0's eps columns are packed as their own dram
tensor so the first transfer is small and fully contiguous.

Startup hiding (the NEFF spends ~6.5us in a fixed semaphore/iram
preamble, then ~4-5us moving the first inputs):
  - The PE's first 8 real matmuls would run ~2x slow (p-state ramp,
    ~3us to full clock), so 8 throwaway DoubleRow matmuls over a
    memset scratch tile ramp the array while the real inputs land.
  - The first Sigmoid pays a ~1.3us ACT_TABLE_LOAD; a throwaway
    activation preloads the table during the head.
mm1 iterates g-outer (first pass needs only half of w1/eps), mm2
oc-outer so bias-add + writeback stream during mm2.
"""

from contextlib import ExitStack

import ml_dtypes
import numpy as np

import concourse.bass as bass
import concourse.tile as tile
from concourse import bacc, mybir
from concourse.bass_utils import run_bass_kernel_spmd

NB_COMP = 8
LAT_DIM = 512
NB_NEUR = 1024
OUT_DIM = 512
N_CORES = 8

F32 = mybir.dt.float32
BF16 = mybir.dt.bfloat16
FP8 = mybir.dt.float8e4
TANH = mybir.ActivationFunctionType.Tanh
DR = mybir.MatmulPerfMode.DoubleRow
MULT = mybir.AluOpType.mult
ADD = mybir.AluOpType.add

E4M3 = ml_dtypes.float8_e4m3
NPBF16 = ml_dtypes.bfloat16
W1_SCALE = 64.0
W2_SCALE = 64.0
N_WARM_MM = 24
K_CAP = 2048  # exactly N_SAMPLES/N_CORES; routing overflow handled on host

KB1 = LAT_DIM // 128   # 4 contraction blocks for mm1
MC1 = NB_NEUR // 128   # 8 output tiles for mm1
KC2 = NB_NEUR // 128   # 8 contraction blocks for mm2
MC2 = OUT_DIM // 128   # 4 output tiles for mm2

_program_cache = {}


def _make_chunks(k_cap):
    """Near-equal chunks, multiples of 16, each <=512 (PSUM bank / moving
    dim limit) and >=256 when possible (full-rate floor). (A small last
    chunk to shorten the writeback tail was tried and measured worse.)"""
    n_chunks = -(-k_cap // 512)
    base = (k_cap // n_chunks) // 16 * 16
    sizes = [base] * n_chunks
    sizes[0] += k_cap - base * n_chunks
    chunks = []
    n0 = 0
    for ns in sizes:
        chunks.append((n0, ns))
        n0 += ns
    return chunks


def _build_program(k_cap):
    """One-expert MLP over k_cap samples; same program runs SPMD on all 8 cores."""
    chunks = _make_chunks(k_cap)
    ns0 = chunks[0][1]
    k_rest = k_cap - ns0

    nc = bacc.Bacc(
        "TRN2",
        target_bir_lowering=False,
        debug=False,
        enable_asserts=False,
        num_devices=N_CORES,
    )
    rchunks = chunks[1:]
    # eps chunk 0 split by DR block pair, w1 split by (g, column half):
    # separate dram tensors + separate SBUF tiles so each first-chunk
    # matmul/ldweights waits only on the one ~128KB DMA it actually
    # needs (Tile dependencies are tile-granular).
    eps0a = nc.dram_tensor("eps0a", [128, 2, ns0], FP8, kind="ExternalInput").ap()
    eps0b = nc.dram_tensor("eps0b", [128, 2, ns0], FP8, kind="ExternalInput").ap()
    epsr = [
        nc.dram_tensor(f"epsr{i}", [128, KB1, ns], FP8, kind="ExternalInput").ap()
        for i, (_, ns) in enumerate(rchunks)
    ]
    w1d = [
        [
            nc.dram_tensor(f"w1_{g}{h}", [128, 2, NB_NEUR // 2], FP8,
                           kind="ExternalInput").ap()
            for h in range(2)
        ]
        for g in range(2)
    ]
    w2 = nc.dram_tensor("w2", [128, KC2, OUT_DIM], FP8, kind="ExternalInput").ap()
    bias = nc.dram_tensor("bias", [128, MC1 + MC2], F32, kind="ExternalInput").ap()
    yT = nc.dram_tensor("yT", [OUT_DIM, k_cap], BF16, kind="ExternalOutput").ap()

    with tile.TileContext(nc) as tc, ExitStack() as ctx:
        wpool = ctx.enter_context(tc.tile_pool(name="weights", bufs=1))
        hpool = ctx.enter_context(tc.tile_pool(name="h", bufs=2))
        ypool = ctx.enter_context(tc.tile_pool(name="y", bufs=10))
        # One shared pool holding all 8 PSUM banks; mm1 keeps 8 accumulators
        # live, mm2 4, cycling through the same slots.
        pspool = ctx.enter_context(tc.tile_pool(name="ps", bufs=8, space="PSUM"))

        w1t = [
            [
                wpool.tile([128, 2, NB_NEUR // 2], FP8, tag=f"w1{g}{h}",
                           name=f"w1t{g}{h}")
                for h in range(2)
            ]
            for g in range(2)
        ]
        x0t = [
            wpool.tile([128, 2, ns0], FP8, tag=f"x0{g}", name=f"x0t{g}")
            for g in range(2)
        ]
        xrt = [
            wpool.tile([128, KB1, ns], FP8, tag=f"xr{i}", name=f"xrt{i}")
            for i, (_, ns) in enumerate(rchunks)
        ]
        bt = wpool.tile([128, MC1 + MC2], F32, tag="bias")
        # w2 as four tiles/DMAs so they land on four different queue
        # quads and drain in parallel.
        w2t = [
            wpool.tile([128, 2, OUT_DIM], FP8, tag=f"w2{i}", name=f"w2t{i}")
            for i in range(4)
        ]
        xwarm = wpool.tile([128, 2, 128], FP8, tag="xwarm")

        # Two parallel DMA ladders. The SP (sync) HWDGE carries only the
        # PE-start critical path -- chunk 0's eps halves and the four w1
        # quarters, in first-use order (each DIRECT2D issue costs ~0.65us
        # serialized on the issuing sequencer, so a single ladder of 14
        # transfers would gate chunk 0's mm2). Everything with a later
        # deadline (bias for the first tanh ~5us after PE start, w2 for
        # chunk 0's mm2, the per-chunk eps remainders) issues from the
        # otherwise-idle gpsimd sequencer, which is past its preamble
        # ~1us before SP.
        # xwarm memset first in the gpsimd stream so the PE warmup isn't
        # stuck behind gpsimd's DMA issues.
        nc.gpsimd.memset(xwarm[:], 0)
        # DMA bandwidth comes from QUEUES: each dma_start stripes all 16
        # SDMA engines but lands on one queue per issuing sequencer, and
        # a queue's FIFO streams at only ~120GB/s. So the startup spread
        # issues across four sequencers = four queues, deadline-ordered
        # within each FIFO:
        #   SP:  eps chunk 0 (gates PE start), then the eps remainders
        #        and the late half of w2 behind them
        #   ACT: w1 g=0 halves (needed with eps0a; 2 issues fit before
        #        its table-load + first tanh ~12us)
        #   gpsimd: w1 g=1 halves (needed at g1 ~13us), then bias + the
        #        early half of w2 gated on eps0b's landing so they don't
        #        contend with the critical window (measured +3-4us on
        #        eps0b without the gate), then y writebacks.
        nc.gpsimd.dma_start(bt[:], bias[:])
        nc.gpsimd.dma_start(w1t[0][0][:], w1d[0][0][:])
        nc.gpsimd.dma_start(w1t[1][0][:], w1d[1][0][:])
        nc.gpsimd.dma_start(x0t[1][:], eps0b[:])
        nc.gpsimd.dma_start(w1t[1][1][:], w1d[1][1][:])
        nc.gpsimd.dma_start(w2t[3][:], w2[:, 6:8, :])
        nc.gpsimd.dma_start(xrt[-1][:], epsr[-1][:])
        nc.sync.dma_start(x0t[0][:], eps0a[:])
        nc.sync.dma_start(w1t[0][1][:], w1d[0][1][:])
        nc.sync.dma_start(w2t[0][:], w2[:, 0:2, :])
        nc.sync.dma_start(w2t[1][:], w2[:, 2:4, :])
        nc.sync.dma_start(w2t[2][:], w2[:, 4:6, :])
        for i in range(len(rchunks) - 1):
            nc.sync.dma_start(xrt[i][:], epsr[i][:])

        # PE p-state warmup: the array runs ~2x slow until ~3.4us of
        # continuous execution, so burn that ramp on garbage DoubleRow
        # matmuls over a memset scratch while the first inputs land
        # (~9.5us). N=128 so the tail quantization is fine-grained; the
        # ramp completes on the first real matmuls.
        pswarm = pspool.tile([128, 512], F32, tag="ps", name="pswarm")
        for i in range(N_WARM_MM):
            nc.tensor.matmul(
                pswarm[:, 0:128], xwarm[:, :, 0:128], xwarm[:], start=True, stop=True,
                perf_mode=DR,
            )
        # Tanh table warmup: the first Tanh pays ~1.3us of
        # ACT_TABLE_LOAD; hide it in the head (emitted after the scalar
        # engine's DMA ladder so it doesn't delay those issues).
        warm = hpool.tile([128, 1], BF16, tag="warm")
        nc.scalar.activation(warm[:], xwarm[:, 0, 0:1], TANH)

        for ci, (n0, ns) in enumerate(chunks):
            if ci == 0:
                def rhs1(g):
                    return x0t[g][:]
            else:
                def rhs1(g, ci=ci, ns=ns):
                    return xrt[ci - 1][:, 2 * g : 2 * g + 2, :]

            def stat1(g, mc):
                return w1t[g][mc // 4][:, :, (mc % 4) * 128 : (mc % 4 + 1) * 128]

            # mm1: fp8 DoubleRow, contraction 512 = 2 groups x (2 blocks
            # packed per cell x 128 partitions). Chunk 0 runs g-outer so
            # its first pass needs only the g=0 half of w1/eps (DMA
            # phasing); later chunks run mc-outer/g-inner so each
            # ps1[mc] stops 2 matmuls in and its tanh starts ~1.7us
            # earlier -- the serial ACT chain (8 x ~580ns, PSUM-read
            # bound) must finish before mm2's last q group or the PE
            # stalls. The tanh goes to bf16 (2 elem/cycle on ACT vs 1
            # for fp8 out); the vector engine then casts pairs into fp8
            # [128, 2, ns] tiles (mc 2q -> slot 0, 2q+1 -> slot 1) that
            # mm2 consumes as DoubleRow moving operands.
            tt = [
                hpool.tile([128, 2, ns], FP8, tag=f"t{q}", name=f"t_{ci}_{q}")
                for q in range(MC1 // 2)
            ]
            ps1 = [
                pspool.tile([128, ns], F32, tag="ps", name=f"ps1_{ci}_{i}")
                for i in range(MC1)
            ]

            def act_cast(mc):
                # fp8 out costs ACT ~40ns/op more than bf16 out, but a
                # bf16 detour through a DVE cast oversubscribed the
                # vector engine (casts + bias-adds > PE chunk time) and
                # added chain latency; direct fp8 keeps ACT ~5us/chunk
                # vs the PE's ~7 and leaves DVE only the bias-adds.
                nc.scalar.activation(
                    tt[mc // 2][:, mc % 2, :], ps1[mc][:], TANH,
                    bias=bt[:, mc : mc + 1], scale=1.0 / (2.0 * W1_SCALE),
                )

            if ci == 0:
                for g in range(2):
                    for mc in range(MC1):
                        nc.tensor.matmul(
                            ps1[mc][:], stat1(g, mc), rhs1(g),
                            start=(g == 0), stop=(g == 1), perf_mode=DR,
                        )
                        if g == 1:
                            act_cast(mc)
            else:
                for mc in range(MC1):
                    for g in range(2):
                        nc.tensor.matmul(
                            ps1[mc][:], stat1(g, mc), rhs1(g),
                            start=(g == 0), stop=(g == 1), perf_mode=DR,
                        )
                    act_cast(mc)

            # mm2: fp8 DoubleRow too (16 matmuls instead of 32 bf16),
            # q-outer with 4 live accumulators so the q-th tanh/cast pair
            # is needed only before the q-th group of 4 matmuls -- the
            # ACT/DVE chain streams just ahead of the PE instead of
            # gating the whole pass. All 4 ps2 stop together at the end;
            # their bias-adds + writebacks overlap the next chunk's mm1.
            last = ci == len(chunks) - 1

            def bias_wb(oc, ps):
                y = ypool.tile([128, ns], BF16, tag="y")
                # bias-adds run on DVE; for the final chunk they are the
                # tail, so alternate DVE/ACT (both idle by then) to halve
                # the serial chain, and split the writeback issues over
                # gpsimd + SP the same way.
                if last and oc % 2:
                    nc.scalar.activation(
                        y[:], ps[:], mybir.ActivationFunctionType.Identity,
                        bias=bt[:, MC1 + oc : MC1 + oc + 1], scale=1.0 / W2_SCALE,
                    )
                else:
                    nc.vector.tensor_scalar(
                        y[:], ps[:], 1.0 / W2_SCALE,
                        bt[:, MC1 + oc : MC1 + oc + 1], MULT, ADD,
                    )
                dst = yT[oc * 128 : (oc + 1) * 128, n0 : n0 + ns]
                # y writebacks issue from gpsimd (idle after the input
                # ladder) so their ~0.6us serialized issue cost never
                # lands on the sync sequencer or the post-last-matmul
                # tail.
                if last and oc % 2:
                    nc.sync.dma_start(dst, y[:])
                else:
                    nc.gpsimd.dma_start(dst, y[:])

            ps2 = [
                pspool.tile([128, ns], F32, tag="ps", name=f"ps2_{ci}_{oc}")
                for oc in range(MC2)
            ]
            # q-outer: the q-th tanh pair is needed only before the
            # q-th group of 4 matmuls, so the ACT chain streams just
            # ahead of the PE. All 4 ps2 stop together at the end;
            # their bias-adds + writebacks overlap the next chunk's
            # mm1 (for the final chunk the bias-adds alternate DVE/ACT
            # and the writebacks alternate gpsimd/SP to halve the tail).
            for q in range(KC2 // 2):
                for oc in range(MC2):
                    nc.tensor.matmul(
                        ps2[oc][:],
                        w2t[q][:, :, oc * 128 : (oc + 1) * 128],
                        tt[q][:],
                        start=(q == 0),
                        stop=(q == KC2 // 2 - 1),
                        perf_mode=DR,
                    )
            for oc in range(MC2):
                bias_wb(oc, ps2[oc])

    nc.compile()
    return nc


def get_program(k_cap):
    if k_cap not in _program_cache:
        _program_cache[k_cap] = _build_program(k_cap)
    return _program_cache[k_cap]


def _softplus(x):
    x = x.astype(np.float64)
    return (np.maximum(x, 0.0) + np.log1p(np.exp(-np.abs(x)))).astype(np.float32)


def _pack_blocks(a, nblk):
    """[nblk*128, C] -> [128, nblk, C] with out[p, b, c] = a[b*128+p, c]."""
    return np.ascontiguousarray(
        a.reshape(nblk, 128, a.shape[1]).transpose(1, 0, 2)
    )


def kernel(epsilon, comp_idx, mu, rho, W1, b1, W2, b2, _trace=False):
    epsilon = np.asarray(epsilon, dtype=np.float32)
    comp_idx = np.asarray(comp_idx, dtype=np.int32)
    mu = np.asarray(mu, dtype=np.float32)
    rho = np.asarray(rho, dtype=np.float32)
    W1 = np.asarray(W1, dtype=np.float32)
    b1 = np.asarray(b1, dtype=np.float32)
    W2 = np.asarray(W2, dtype=np.float32)
    b2 = np.asarray(b2, dtype=np.float32)

    n = epsilon.shape[0]
    sigma = _softplus(rho)  # [C]

    # Each core runs exactly K_CAP = N/8 samples of its expert (4 uniform
    # 512-sample chunks). Experts routed more than K_CAP samples overflow
    # the excess to an exact host-side computation -- the device always
    # does its fair 1/8 share; the host only absorbs routing imbalance
    # (~0.4% of samples for a balanced router).
    sels_all = [np.nonzero(comp_idx == c)[0] for c in range(NB_COMP)]
    sels = [s[:K_CAP] for s in sels_all]
    overflow = [s[K_CAP:] for s in sels_all]
    k_cap = K_CAP

    nc = get_program(k_cap)
    chunks = _make_chunks(k_cap)
    ns0 = chunks[0][1]

    eps_q = epsilon.astype(E4M3)  # quantize once; std ~1 sits mid e4m3 range
    in_maps = []
    for c in range(NB_COMP):
        sel = sels[c]
        epsT = np.zeros((128, KB1, k_cap), dtype=E4M3)
        if len(sel):
            epsT[:, :, : len(sel)] = _pack_blocks(eps_q[sel].T, KB1)
        w1p = (W1[c] * (sigma[c] * W1_SCALE)).astype(E4M3)
        # device computes t = tanh(x/2); activation bias must be b1p/2
        b1p_half = 0.5 * (
            b1[c].astype(np.float64) + mu[c].astype(np.float64) @ W1[c].astype(np.float64)
        )
        # y = sum_k W2[k,o] * (0.5 + 0.5*t_k) + b2 = ps2/W2_SCALE + bias2
        # with the stationary operand quantized as 0.5*W2*W2_SCALE
        w2p = (W2[c] * (0.5 * W2_SCALE)).astype(E4M3)
        bias2 = b2[c].astype(np.float64) + 0.5 * W2[c].astype(np.float64).sum(axis=0)
        bias_c = np.concatenate(
            [
                b1p_half.astype(np.float32).reshape(MC1, 128).T,
                bias2.astype(np.float32).reshape(MC2, 128).T,
            ],
            axis=1,
        )
        w1P = _pack_blocks(w1p, KB1)  # [128, 4, 1024]
        im = {
            "eps0a": np.ascontiguousarray(epsT[:, 0:2, :ns0]),
            "eps0b": np.ascontiguousarray(epsT[:, 2:4, :ns0]),
            "w2": _pack_blocks(w2p, KC2),
            "bias": np.ascontiguousarray(bias_c),
        }
        for g in range(2):
            for h in range(2):
                im[f"w1_{g}{h}"] = np.ascontiguousarray(
                    w1P[:, 2 * g : 2 * g + 2, 512 * h : 512 * (h + 1)]
                )
        for i, (cn0, cns) in enumerate(chunks[1:]):
            im[f"epsr{i}"] = np.ascontiguousarray(epsT[:, :, cn0 : cn0 + cns])
        in_maps.append(im)

    res = run_bass_kernel_spmd(
        nc,
        in_maps,
        core_ids=list(range(N_CORES)),
        trace=_trace,
        trace_cores=list(range(N_CORES)) if _trace else None,
    )

    out = np.zeros((n, OUT_DIM), dtype=np.float32)
    for c in range(NB_COMP):
        sel = sels[c]
        if len(sel):
            out[sel] = res.results[c]["yT"][:, : len(sel)].T.astype(np.float32)
        ov = overflow[c]
        if len(ov):
            lat = epsilon[ov] * sigma[c] + mu[c]
            x = lat @ (W1[c]) + b1[c]
            hh = 1.0 / (1.0 + np.exp(-x))
            out[ov] = hh @ W2[c] + b2[c]
    if _trace:
        return out, res
    return out

